# revision 3
# baseline (speedup 1.0000x reference)
"""Causal single-head attention (Q==K source bug faithful) on 8 TRN2 NeuronCores.

Problem: x [4, 4096, 1024], Wk/Wv [1024, 64];
  k = q = x@Wk; scores = q k^T / 8, causal softmax, out = weights @ (x@Wv).

v6 strategy — v5's balanced key-split plus PE row-packing:
  - 8 cores = 4 batches x 2 parities; core parity r owns half the key
    blocks of each 1024-query chunk (positions (j+r)%2 + {0,2,4,6}).
    Each core emits per-chunk partial softmax accumulators
    [65, 1024] = (V|1)^T exp(S^T); host adds the two partials / divides.
  - Scores matmuls have contraction 64 (head dim) — half the PE rows
    idle. v6 packs TWO key blocks per slot via tile_position row
    tiling: K^T of block A at partitions 0-63, block B at partitions
    64-127 (kt holds Q^T duplicated on both halves), so both 128x512
    score matmuls stream concurrently: slot stream time halves.
  - Q^T duplication is free for peer panels ([Wk|Wk] weight) and a
    SBUF->SBUF DMA for own panels.
  - Diagonal slots: B streams from its own causal offset; masks are a
    single [128,2,256] table multiply (own tril / peer parity).
  - HAM warm-up: ~10 dummy matmuls on const data while input DMAs
    land, so real matmuls start at 2.4 GHz instead of 1.2.
  - exp on ScalarE (scale=1/8 fused) from PSUM; output partial copies
    on Pool; projection copies + masks on DVE; two DMA queues with
    first panels split for an early start.
"""
import numpy as np
import ml_dtypes

import concourse.bass as bass
import concourse.mybir as mybir
from concourse import bacc, tile
from concourse.bass_utils import run_bass_kernel_spmd

F32 = mybir.dt.float32
BF16 = mybir.dt.bfloat16
EXP = mybir.ActivationFunctionType.Exp

B, T, C, H = 4, 4096, 1024, 64
NCHI = C // 128          # 8 contraction blocks
NPAN = 8                 # panels (0..3 own, 4..7 peer), 512 rows each
PAN = 512
CHUNK = 1024             # queries per chunk
NCK = T // CHUNK         # 4 chunks

# const blob layout (bf16 cols per partition)
CW_WKV = 1024            # [8, 128] own weights  [Wk | Wv]
CW_WKK = 1024            # [8, 128] peer weights [Wk | Wk]
CW_MSK = 1536            # [3, 2, 256] masks: 0=own tril, 1=peer jpar0, 2=peer jpar1
CW_EYE = 65              # eye at [64:128, 0:64], ones col 64
CSTW = CW_WKV + CW_WKK + CW_MSK + CW_EYE


def build_nc():
    nc = bacc.Bacc("TRN2", target_bir_lowering=False, debug=False, num_devices=8)

    xt_d = nc.declare_dram_parameter("xt", [NPAN, 128, NCHI, PAN], BF16, isOutput=False)
    cst_d = nc.declare_dram_parameter("cst", [128, CSTW], BF16, isOutput=False)
    out_d = nc.declare_dram_parameter("out", [NCK, 65, 1024], F32, isOutput=True)

    with tile.TileContext(nc) as tc:
        with (
            tc.tile_pool(name="const", bufs=1) as const,
            tc.tile_pool(name="xt", bufs=NPAN) as xtp,
            tc.tile_pool(name="vsb", bufs=2) as vsbp,
            tc.tile_pool(name="pt", bufs=6) as ptp,
            tc.tile_pool(name="osb", bufs=2) as osbp,
            tc.tile_pool(name="psA", bufs=2, space="PSUM") as psA,
            tc.tile_pool(name="psO", bufs=2, space="PSUM") as psO,
        ):
            cst = const.tile([128, CSTW], BF16, tag="cst")
            wkv = cst[:, 0:CW_WKV].rearrange("p (a b) -> p a b", a=NCHI)
            wkk = cst[:, CW_WKV:CW_WKV + CW_WKK].rearrange("p (a b) -> p a b", a=NCHI)
            mo = CW_WKV + CW_WKK
            msk = cst[:, mo:mo + CW_MSK].rearrange("p (a b c) -> p a b c", a=3, b=2)
            eyeb = cst[:, mo + CW_MSK:mo + CW_MSK + CW_EYE]

            kt = const.tile([128, T], BF16, tag="kt")       # rows 0-63 K^T, 64-127 dup
            vaug = const.tile([128, 16, 128], BF16, tag="vaug")  # V|1|0pad per own kb

            # DMA queue G (gpsimd): consts + own panels; S (sync): peer panels.
            nc.gpsimd.dma_start(cst[:], cst_d[:])
            xts = []
            for p in range(NPAN):
                xt = xtp.tile([128, NCHI, PAN], BF16, tag="xt")
                q = nc.gpsimd if p < 4 else nc.sync
                if p in (0, 4):  # split first panels for an early start
                    q.dma_start(xt[:, 0:4, :], xt_d[p][:, 0:4, :])
                    q.dma_start(xt[:, 4:8, :], xt_d[p][:, 4:8, :])
                else:
                    q.dma_start(xt[:], xt_d[p])
                xts.append(xt)

            # vaug: ones col 64, zero cols 65..127 (FWL padding)
            nc.vector.tensor_copy(
                vaug[:, :, 64:65],
                eyeb[:, 64:65].unsqueeze(1).broadcast_to([128, 16, 1]),
            )
            nc.gpsimd.memset(vaug[:, :, 65:128], 0)

            for _ in range(2):
                z = psA.tile([128, 1024], F32, tag="ps", name="z")
                nc.scalar.memzero(z[:])

            # HAM warm-up: dummy matmuls on const data while input DMAs land.
            wps = psA.tile([128, 1024], F32, tag="ps", name="warm")
            for _ in range(10):
                nc.tensor.matmul(
                    wps[0:65, 0:512], eyeb[:, 0:65], cst[:, 0:512],
                    start=True, stop=True,
                )

            def proj_own(p):
                xt = xts[p]
                pj = psA.tile([128, 1024], F32, tag="ps")
                for ci in range(NCHI):
                    nc.tensor.matmul(
                        pj[:, 0:PAN], wkv[:, ci, :], xt[:, ci, :],
                        start=(ci == 0), stop=(ci == NCHI - 1),
                    )
                rng = slice(p * PAN, (p + 1) * PAN)
                nc.vector.tensor_copy(kt[0:64, rng], pj[0:64, 0:PAN])
                # duplicate K^T onto partitions 64-127 (row-packed rhs/lhsT)
                nc.sync.dma_start(kt[64:128, rng], kt[0:64, rng])
                vsb = vsbp.tile([128, PAN], BF16, tag="vsb")
                nc.vector.tensor_copy(vsb[64:128, :], pj[64:128, 0:PAN])
                v_ps = pj[:, PAN:PAN + 128].bitcast(BF16).rearrange(
                    "p (a b) -> p a b", a=4)
                for tb in range(4):
                    nc.tensor.transpose(
                        v_ps[:, tb, :], vsb[64:128, tb * 128:(tb + 1) * 128],
                        eyeb[64:128, 0:64],
                    )
                nc.vector.tensor_copy(vaug[:, 4 * p:4 * p + 4, 0:64], v_ps[:])

            def proj_peer(p):
                xt = xts[p]
                pj = psA.tile([128, 1024], F32, tag="ps")
                for ci in range(NCHI):
                    nc.tensor.matmul(
                        pj[:, 0:PAN], wkk[:, ci, :], xt[:, ci, :],
                        start=(ci == 0), stop=(ci == NCHI - 1),
                    )
                rng = slice(2048 + (p - 4) * PAN, 2048 + (p - 3) * PAN)
                nc.vector.tensor_copy(kt[:, rng], pj[:, 0:PAN])

            ot = {}       # live chunk accumulators [128, 1024]
            started = {}  # (j, rng) -> bank already started

            def scores_exp(j, kA, kB, c0, rng, diag):
                """Pair-slot front half: two row-packed score matmuls
                (block A at PE rows 0-63, B at rows 64-127) + one exp.
                rng: 0=own half queries, 1=peer half. diag slots trim
                A to [c0:] and B to [c0+128:] and mask via msk table."""
                sp = psA.tile([128, 1024], F32, tag="ps")
                spv = sp.rearrange("p (b c) -> p b c", b=2)
                qb = 2048 * rng + j * PAN
                bA = c0
                bB = c0 + 128 if diag else 0
                nc.tensor.matmul(
                    spv[:, 0, bA:512],
                    kt[0:64, kA * 128:(kA + 1) * 128], kt[0:64, qb + bA:qb + 512],
                    start=True, stop=True,
                )
                nc.tensor.matmul(
                    spv[:, 1, bB:512],
                    kt[64:128, kB * 128:(kB + 1) * 128], kt[64:128, qb + bB:qb + 512],
                    start=True, stop=True,
                )
                pt = ptp.tile([128, 2, 512], BF16, tag="pt")
                nc.scalar.activation(pt[:, :, c0:512], spv[:, :, c0:512], EXP, scale=0.125)
                if diag:
                    mi = 0 if rng == 0 else 1 + (j % 2)
                    nc.vector.tensor_mul(
                        pt[:, :, c0:c0 + 256], pt[:, :, c0:c0 + 256], msk[:, mi]
                    )
                return pt

            def pv(j, kA, kB, pt, c0, rng, diag, last):
                acc = ot[j]
                off = 512 * rng
                bA = c0
                bB = c0 + 128 if diag else 0
                firstA = not started.get((j, rng), False)
                started[(j, rng)] = True
                nc.tensor.matmul(
                    acc[:, off + bA:off + 512], vaug[:, kA, :], pt[:, 0, bA:512],
                    start=firstA, stop=last, skip_group_check=True,
                )
                nc.tensor.matmul(
                    acc[:, off + bB:off + 512], vaug[:, kB, :], pt[:, 1, bB:512],
                    start=False, stop=last, skip_group_check=True,
                )

            def chunk_slots(j):
                s = []
                for jj in range(j):
                    for m in (0, 1):
                        for rng in (0, 1):
                            s.append(("slot", dict(
                                j=j, kA=4 * jj + 2 * m, kB=4 * jj + 2 * m + 1,
                                c0=0, rng=rng, diag=False)))
                for m in (0, 1):
                    for rng in (0, 1):
                        s.append(("slot", dict(
                            j=j, kA=4 * j + 2 * m, kB=4 * j + 2 * m + 1,
                            c0=256 * m, rng=rng, diag=True,
                            last=(m == 1 and rng == 1))))
                return s

            actions = [("proj_own", 0), ("proj_peer", 4)]
            actions += chunk_slots(0) + [("out", 0)]
            actions += [("proj_own", 1), ("proj_peer", 5)]
            actions += chunk_slots(1) + [("out", 1)]
            actions += [("proj_own", 2), ("proj_peer", 6),
                        ("proj_own", 3), ("proj_peer", 7)]
            c2, c3 = chunk_slots(2), chunk_slots(3)
            mix = []
            while c2 or c3:
                if c2:
                    mix.append(c2.pop(0))
                if c3:
                    mix.append(c3.pop(0))
                if c3:
                    mix.append(c3.pop(0))
            for a in mix:
                actions.append(a)
                if a[1].get("last") and a[1]["j"] == 2:
                    actions.append(("out", 2))
            actions.append(("out", 3))

            pending = None

            def flush():
                nonlocal pending
                if pending is not None:
                    a, pt = pending
                    if a["j"] not in ot:
                        ot[a["j"]] = psO.tile([128, 1024], F32, tag="ot", name="ot")
                    pv(a["j"], a["kA"], a["kB"], pt, a["c0"], a["rng"],
                       a["diag"], a.get("last", False))
                    pending = None

            for kind, arg in actions:
                if kind == "proj_own":
                    proj_own(arg)
                elif kind == "proj_peer":
                    proj_peer(arg)
                elif kind == "slot":
                    if arg["j"] not in ot:
                        ot[arg["j"]] = psO.tile([128, 1024], F32, tag="ot", name="ot")
                    pt = scores_exp(arg["j"], arg["kA"], arg["kB"],
                                    arg["c0"], arg["rng"], arg["diag"])
                    flush()
                    pending = (arg, pt)
                else:  # out: partials via an SBUF bounce
                    flush()
                    osb = osbp.tile([65, 1024], F32, tag="osb")
                    nc.vector.tensor_copy(osb[:], ot[arg][0:65, :])
                    nc.sync.dma_start(out_d.ap()[arg], osb[:])

    nc.compile()
    return nc


def _own_blocks(j, r):
    """Global 128-row key-block positions (within chunk j) owned by
    parity r, in local order."""
    q = (j + r) % 2
    return [q + 2 * i for i in range(4)]


def make_inputs(x, Wk, Wv):
    """Build the 8 per-core input maps (pure layout work)."""
    bf16 = ml_dtypes.bfloat16
    wkv = np.concatenate([Wk, Wv], axis=1)            # [1024, 128]
    wkv_t = np.ascontiguousarray(
        wkv.reshape(NCHI, 128, 128).transpose(1, 0, 2)
    ).astype(bf16)  # [cp, chi, m]
    wkk = np.concatenate([Wk, Wk], axis=1)
    wkk_t = np.ascontiguousarray(
        wkk.reshape(NCHI, 128, 128).transpose(1, 0, 2)
    ).astype(bf16)

    eyeb = np.zeros((128, 65), dtype=np.float32)
    eyeb[64:128, 0:64] = np.eye(64)
    eyeb[:, 64] = 1.0

    kk = np.arange(128)[:, None]                      # key row within block
    cc = np.arange(256)[None, :]                      # window column

    # msk[*, 0]: own-range diag mask. A: tril then ones; B: zeros then tril.
    gm2 = np.empty((128, 2, 256), dtype=np.float32)
    gm2[:, 0, :] = np.concatenate(
        [(cc[:, 0:128] >= kk), np.ones((128, 128))], axis=1)
    gm2[:, 1, :] = np.concatenate(
        [np.zeros((128, 128)), (cc[:, 0:128] >= kk)], axis=1)

    def peer_mask(qpar):
        pm = np.ones((128, 2, 256), dtype=np.float32)
        if qpar == 1:
            pm[:, 0, 0:128] = 0.0
            pm[:, 1, :] = 0.0
        return pm

    in_maps = []
    for c in range(8):
        b, r = c % 4, c // 4

        xT = np.ascontiguousarray(x[b].T)             # [1024, 4096]
        xr = xT.reshape(NCHI, 128, T)                 # [chi, cp, t]
        xt = np.empty((NPAN, 128, NCHI, PAN), dtype=bf16)
        for p in range(NPAN):
            j, rr = (p, r) if p < 4 else (p - 4, 1 - r)
            rows = np.concatenate([
                np.arange(j * CHUNK + m * 128, j * CHUNK + m * 128 + 128)
                for m in _own_blocks(j, rr)
            ])
            xt[p] = xr[:, :, rows].transpose(1, 0, 2)

        cst = np.empty((128, CSTW), dtype=bf16)
        cst[:, 0:CW_WKV] = wkv_t.reshape(128, -1)
        cst[:, CW_WKV:CW_WKV + CW_WKK] = wkk_t.reshape(128, -1)
        mo = CW_WKV + CW_WKK
        mtab = np.stack([gm2, peer_mask(r % 2), peer_mask((1 + r) % 2)], axis=1)
        cst[:, mo:mo + CW_MSK] = mtab.reshape(128, -1).astype(bf16)
        cst[:, mo + CW_MSK:] = eyeb.astype(bf16)

        in_maps.append({"xt": xt, "cst": cst})
    return in_maps


_NC = None


def get_nc():
    global _NC
    if _NC is None:
        _NC = build_nc()
    return _NC


def kernel(x, Wk, Wv):
    x = np.asarray(x, dtype=np.float32)
    Wk = np.asarray(Wk, dtype=np.float32)
    Wv = np.asarray(Wv, dtype=np.float32)
    nc = get_nc()
    in_maps = make_inputs(x, Wk, Wv)
    res = run_bass_kernel_spmd(nc, in_maps, list(range(8)))

    out = np.empty((B, T, H), dtype=np.float32)
    for b in range(4):
        p0 = res.results[b]["out"].astype(np.float64)      # parity 0
        p1 = res.results[b + 4]["out"].astype(np.float64)  # parity 1
        for j in range(NCK):
            tot = np.zeros((65, 1024), dtype=np.float64)
            for r, part in ((0, p0[j]), (1, p1[j])):
                blocks = _own_blocks(j, r) + _own_blocks(j, 1 - r)
                glob = np.empty((65, 1024), dtype=np.float64)
                for k, m in enumerate(blocks):
                    glob[:, m * 128:(m + 1) * 128] = part[:, k * 128:(k + 1) * 128]
                tot += glob
            out[b, j * CHUNK:(j + 1) * CHUNK] = (tot[0:64] / tot[64]).T
    return out
